# Initial kernel scaffold
#
"""Trainium2 Bass kernel for ChemiNet-style NNConv GNN (8 NeuronCores).

Math restructure: the final output per graph g is
    out[g] = sum_{e: dst in g} m1[e] + sum_{n in g} hx[n] + cnt_g*cb1 + ob
with
    m1[e] = sum_{i,o} ow[o] * x[src_e,i] * relu(q[e, i, o])
    q[e]  = edge_attr_aug[e] @ Wfold          (bias folded as 13th input)
    hx[n] = x[n] . (root_w @ ow)
    cb1   = conv_b . ow,  ob = out_b[0]
|ow_o| is folded into Wfold's rows; the sign of ow_o is handled by splitting
columns into a positive block multiplied by +x and a negative block
multiplied by -x (x*relu(w) = relu(|c|*w)*sign pattern via column grouping).

Device work per 128-edge tile: one PE matmul (K=13) into PSUM plus two fused
DVE scalar_tensor_tensor ops (relu+mul+row-sum in one pass) and one ScalarE
add. Sharding: graphs split into 8 contiguous ranges balanced by edge count;
each core gets its edges (sorted by dst graph) and its node range.
"""

import numpy as np

F_ATOM = 75
F_BOND = 12
OUT = 10
G_TOTAL = 2048
N_CORES = 8
P = 128           # partitions
ST = 16           # tiles per DMA/output group
EDGE_GRAN = P * ST
NODE_GRAN = P * ST

_PROG_CACHE = {}


def _build_program(e_tiles, n_tiles, kp, kn):
    import concourse.bass as bass
    import concourse.mybir as mybir
    import concourse.tile as tile

    f32 = mybir.dt.float32
    nc = bass.Bass()

    EPC = e_tiles * P
    NPC = n_tiles * P
    eaT = nc.declare_dram_parameter("eaT", [F_BOND + 1, EPC], f32, isOutput=False)
    xe = nc.declare_dram_parameter("xe", [EPC, 2 * F_ATOM], f32, isOutput=False)
    xs = nc.declare_dram_parameter("xs", [NPC, F_ATOM], f32, isOutput=False)
    Wf = nc.declare_dram_parameter("Wf", [F_BOND + 1, F_ATOM * OUT], f32, isOutput=False)
    rw1 = nc.declare_dram_parameter("rw1", [P, F_ATOM], f32, isOutput=False)
    m1o = nc.declare_dram_parameter("m1o", [e_tiles // ST, P, ST], f32, isOutput=True)
    hxo = nc.declare_dram_parameter("hxo", [n_tiles // ST, P, ST], f32, isOutput=True)

    COLS = F_ATOM * OUT          # 750
    KPW = F_ATOM * kp            # width of positive block
    KNW = F_ATOM * kn

    mul = mybir.AluOpType.mult
    add = mybir.AluOpType.add
    mx = mybir.AluOpType.max

    with tile.TileContext(nc) as tc:
        with (
            tc.tile_pool(name="const", bufs=1) as cp,
            tc.tile_pool(name="ea", bufs=3) as eap,
            tc.tile_pool(name="xed", bufs=3) as xep,
            tc.tile_pool(name="ps", bufs=2, space="PSUM") as psp,
            tc.tile_pool(name="scr", bufs=2) as scrp,
            tc.tile_pool(name="strip", bufs=2) as stp,
            tc.tile_pool(name="acc", bufs=2) as accp,
        ):
            Wt = cp.tile([F_BOND + 1, COLS], f32)
            nc.sync.dma_start(Wt[:], Wf[:])
            rt = cp.tile([P, F_ATOM], f32)
            nc.sync.dma_start(rt[:], rw1[:])

            # ---- node phase: hx[n] = x[n] . rw1 ----
            for g in range(n_tiles // ST):
                hxs = stp.tile([P, ST], f32, tag="hxs")
                xsl = xep.tile([P, ST * F_ATOM], f32, tag="xsl")
                src = xs[g * NODE_GRAN:(g + 1) * NODE_GRAN, :].rearrange(
                    "(t p) f -> p (t f)", p=P)
                nc.sync.dma_start(xsl[:], src)
                for c in range(ST):
                    so = scrp.tile([P, F_ATOM], f32, tag="so")
                    nc.vector.tensor_tensor_reduce(
                        out=so[:],
                        in0=xsl[:, c * F_ATOM:(c + 1) * F_ATOM],
                        in1=rt[:],
                        scale=1.0,
                        scalar=0.0,
                        op0=mul,
                        op1=add,
                        accum_out=hxs[:, c:c + 1],
                    )
                nc.sync.dma_start(hxo[g], hxs[:])

            # ---- edge phase ----
            for g in range(e_tiles // ST):
                m1s = stp.tile([P, ST], f32, tag="m1s")
                esl = eap.tile([F_BOND + 1, ST * P], f32, tag="esl")
                nc.sync.dma_start(esl[:], eaT[:, g * EDGE_GRAN:(g + 1) * EDGE_GRAN])
                xesl = xep.tile([P, ST * 2 * F_ATOM], f32, tag="xesl")
                xsrc = xe[g * EDGE_GRAN:(g + 1) * EDGE_GRAN, :].rearrange(
                    "(t p) f -> p (t f)", p=P)
                nc.sync.dma_start(xesl[:], xsrc)
                for c in range(ST):
                    ea_t = esl[:, c * P:(c + 1) * P]
                    xe_t = xesl[:, c * 2 * F_ATOM:(c + 1) * 2 * F_ATOM]
                    q = psp.tile([P, 768], f32, tag="q")
                    nc.tensor.matmul(q[:, 0:512], ea_t, Wt[:, 0:512],
                                     start=True, stop=True)
                    nc.tensor.matmul(q[:, 512:COLS], ea_t, Wt[:, 512:COLS],
                                     start=True, stop=True)
                    po = scrp.tile([P, COLS], f32, tag="po")
                    m1a = accp.tile([P, 1], f32, tag="m1a")
                    m1b = accp.tile([P, 1], f32, tag="m1b")
                    if kp > 0:
                        nc.vector.scalar_tensor_tensor(
                            out=po[:, 0:KPW].rearrange("p (i o) -> p i o", o=kp),
                            in0=q[:, 0:KPW].rearrange("p (i o) -> p i o", o=kp),
                            scalar=0.0,
                            in1=xe_t[:, 0:F_ATOM].rearrange("p i -> p i 1")
                                .broadcast_to([P, F_ATOM, kp]),
                            op0=mx,
                            op1=mul,
                            accum_out=m1a[:],
                        )
                    else:
                        nc.vector.memset(m1a[:], 0.0)
                    if kn > 0:
                        nc.vector.scalar_tensor_tensor(
                            out=po[:, KPW:COLS].rearrange("p (i o) -> p i o", o=kn),
                            in0=q[:, KPW:COLS].rearrange("p (i o) -> p i o", o=kn),
                            scalar=0.0,
                            in1=xe_t[:, F_ATOM:2 * F_ATOM].rearrange("p i -> p i 1")
                                .broadcast_to([P, F_ATOM, kn]),
                            op0=mx,
                            op1=mul,
                            accum_out=m1b[:],
                        )
                    else:
                        nc.vector.memset(m1b[:], 0.0)
                    nc.scalar.activation(
                        m1s[:, c:c + 1], m1a[:],
                        mybir.ActivationFunctionType.Copy,
                        bias=m1b[:], scale=1.0)
                nc.sync.dma_start(m1o[g], m1s[:])
    return nc


def _prep(x, edge_index, edge_attr, batch, lin_w, lin_b, root_w, conv_b,
          out_w, out_b, G):
    """Host-side sharding + weight folding. Returns per-core input maps and
    metadata for the final combine."""
    E = edge_index.shape[1]
    N = x.shape[0]

    src = edge_index[0].astype(np.int64)
    dst = edge_index[1].astype(np.int64)
    ge = batch[dst]                       # graph of each edge's destination
    perm = np.argsort(ge, kind="stable")
    ge_s = ge[perm]
    src_s = src[perm]
    ea_s = edge_attr[perm]

    ecnt = np.bincount(ge_s, minlength=G)
    ecum = np.concatenate([[0], np.cumsum(ecnt)])
    ncnt = np.bincount(batch, minlength=G)
    ncum = np.concatenate([[0], np.cumsum(ncnt)])

    # split graphs into N_CORES contiguous ranges, balanced by edge count
    gb = [0]
    for c in range(1, N_CORES):
        gb.append(int(np.searchsorted(ecum[1:], E * c / N_CORES)))
    gb.append(G)
    gb = np.array(gb)

    e_rngs = [(int(ecum[gb[c]]), int(ecum[gb[c + 1]])) for c in range(N_CORES)]
    n_rngs = [(int(ncum[gb[c]]), int(ncum[gb[c + 1]])) for c in range(N_CORES)]

    max_e = max(e1 - e0 for e0, e1 in e_rngs)
    max_n = max(n1 - n0 for n0, n1 in n_rngs)
    EPC = -(-max_e // EDGE_GRAN) * EDGE_GRAN
    NPC = -(-max_n // NODE_GRAN) * NODE_GRAN

    # weight folding: |ow| into rows, sign via column blocks, i-major o-minor
    ow = out_w.reshape(-1).astype(np.float64)
    o_pos = np.where(ow >= 0)[0]
    o_neg = np.where(ow < 0)[0]
    kp, kn = len(o_pos), len(o_neg)
    o_order = np.concatenate([o_pos, o_neg]).astype(np.int64)
    # column j of block: (i, o) i-major within each sign block
    i_idx = np.repeat(np.arange(F_ATOM), kp)
    o_idx = np.tile(o_pos, F_ATOM)
    rows_p = i_idx * OUT + o_idx
    i_idx = np.repeat(np.arange(F_ATOM), kn)
    o_idx = np.tile(o_neg, F_ATOM)
    rows_n = i_idx * OUT + o_idx
    rows = np.concatenate([rows_p, rows_n])
    absow = np.abs(ow)[np.concatenate([np.tile(o_pos, F_ATOM),
                                       np.tile(o_neg, F_ATOM)])]
    Wcols = lin_w[rows].astype(np.float64) * absow[:, None]          # [750,12]
    bcols = lin_b[rows].astype(np.float64) * absow                   # [750]
    Wf = np.concatenate([Wcols, bcols[:, None]], axis=1).T           # [13,750]
    Wf = np.ascontiguousarray(Wf, dtype=np.float32)

    rw1 = (root_w.astype(np.float64) @ ow).astype(np.float32)        # [75]
    rw1_rep = np.ascontiguousarray(np.broadcast_to(rw1[None, :], (P, F_ATOM)),
                                   dtype=np.float32)

    in_maps = []
    for c in range(N_CORES):
        e0, e1 = e_rngs[c]
        ne = e1 - e0
        eaT = np.zeros((F_BOND + 1, EPC), dtype=np.float32)
        eaT[:F_BOND, :ne] = ea_s[e0:e1].T
        eaT[F_BOND, :ne] = 1.0
        xsrc = x[src_s[e0:e1]].astype(np.float32)
        xef = np.zeros((EPC, 2 * F_ATOM), dtype=np.float32)
        xef[:ne, :F_ATOM] = xsrc
        xef[:ne, F_ATOM:] = -xsrc
        n0, n1 = n_rngs[c]
        nn = n1 - n0
        xsf = np.zeros((NPC, F_ATOM), dtype=np.float32)
        xsf[:nn] = x[n0:n1]
        in_maps.append({
            "eaT": eaT, "xe": xef, "xs": xsf, "Wf": Wf, "rw1": rw1_rep,
        })

    cb1 = float(np.dot(conv_b.astype(np.float64), ow))
    ob = float(np.asarray(out_b).reshape(-1)[0])
    meta = dict(gb=gb, e_rngs=e_rngs, n_rngs=n_rngs, ge_s=ge_s, batch=batch,
                ncnt=ncnt, cb1=cb1, ob=ob, EPC=EPC, NPC=NPC, kp=kp, kn=kn)
    return in_maps, meta


def _combine(results, meta, G):
    gb = meta["gb"]
    out = np.zeros(G, dtype=np.float64)
    for c in range(N_CORES):
        g0, g1 = int(gb[c]), int(gb[c + 1])
        e0, e1 = meta["e_rngs"][c]
        n0, n1 = meta["n_rngs"][c]
        m1 = results[c]["m1o"].transpose(0, 2, 1).reshape(-1)[:e1 - e0]
        hx = results[c]["hxo"].transpose(0, 2, 1).reshape(-1)[:n1 - n0]
        esum = np.bincount(meta["ge_s"][e0:e1] - g0, weights=m1,
                           minlength=g1 - g0)
        nsum = np.bincount(meta["batch"][n0:n1] - g0, weights=hx,
                           minlength=g1 - g0)
        out[g0:g1] = esum + nsum
    out += meta["ncnt"] * meta["cb1"] + meta["ob"]
    return out.astype(np.float32)[:, None]


def kernel(x, edge_index, edge_attr, batch, lin_w, lin_b, root_w, conv_b,
           out_w, out_b, num_graphs, _trace=False):
    from concourse.bass_utils import run_bass_kernel_spmd

    x = np.asarray(x, dtype=np.float32)
    edge_index = np.asarray(edge_index)
    edge_attr = np.asarray(edge_attr, dtype=np.float32)
    batch = np.asarray(batch).astype(np.int64)
    lin_w = np.asarray(lin_w, dtype=np.float32)
    lin_b = np.asarray(lin_b, dtype=np.float32)
    root_w = np.asarray(root_w, dtype=np.float32)
    conv_b = np.asarray(conv_b, dtype=np.float32)
    out_w = np.asarray(out_w, dtype=np.float32)
    out_b = np.asarray(out_b, dtype=np.float32)
    G = int(num_graphs)

    in_maps, meta = _prep(x, edge_index, edge_attr, batch, lin_w, lin_b,
                          root_w, conv_b, out_w, out_b, G)

    key = (meta["EPC"] // P, meta["NPC"] // P, meta["kp"], meta["kn"])
    if key not in _PROG_CACHE:
        _PROG_CACHE[key] = _build_program(*key)
    nc = _PROG_CACHE[key]

    res = run_bass_kernel_spmd(nc, in_maps, list(range(N_CORES)),
                               trace=_trace)
    out = _combine(res.results, meta, G)
    if _trace:
        return out, res
    return out


# revision 6
# speedup vs baseline: 2021.8357x; 2021.8357x over previous
"""Trainium2 Bass kernel for ChemiNet-style NNConv GNN (8 NeuronCores).

Math restructure: the final output per graph g is
    out[g] = sum_{e: dst in g} m1[e] + sum_{n in g} hx[n] + cnt_g*cb1 + ob
with
    m1[e] = sum_{i,o} ow[o] * x[src_e,i] * relu(q[e, i, o])
    q[e]  = edge_attr_aug[e] @ Wfold          (bias folded as 13th input)
    hx[n] = x[n] . (root_w @ ow)
    cb1   = conv_b . ow,  ob = out_b[0]
|ow_o| is folded into Wfold's rows; the sign of ow_o is handled by splitting
columns into a positive block multiplied by +x and a negative block
multiplied by -x (x*relu(w) = relu(|c|*w)*sign pattern via column grouping).

Device work per 128-edge tile: one PE matmul (K=13) into PSUM plus two fused
DVE scalar_tensor_tensor ops (relu+mul+row-sum in one pass) and one ScalarE
add. Sharding: graphs split into 8 contiguous ranges balanced by edge count;
each core gets its edges (sorted by dst graph) and its node range.
"""

import numpy as np

F_ATOM = 75
F_BOND = 12
OUT = 10
G_TOTAL = 2048
N_CORES = 8
P = 128           # partitions
ST = 16           # tiles per DMA/output group
EDGE_GRAN = P * ST
NODE_GRAN = P * ST

_PROG_CACHE = {}


def _build_program(e_tiles, n_tiles, kp, kn):
    import concourse.bass as bass
    import concourse.mybir as mybir
    import concourse.tile as tile

    f32 = mybir.dt.float32
    nc = bass.Bass()

    EPC = e_tiles * P
    NPC = n_tiles * P
    eaT = nc.declare_dram_parameter("eaT", [F_BOND + 1, EPC], f32, isOutput=False)
    xe = nc.declare_dram_parameter("xe", [EPC, 2 * F_ATOM], f32, isOutput=False)
    xs = nc.declare_dram_parameter("xs", [NPC, F_ATOM], f32, isOutput=False)
    Wf = nc.declare_dram_parameter("Wf", [F_BOND + 1, F_ATOM * OUT], f32, isOutput=False)
    rw1 = nc.declare_dram_parameter("rw1", [P, F_ATOM], f32, isOutput=False)
    m1o = nc.declare_dram_parameter("m1o", [e_tiles // ST, P, ST], f32, isOutput=True)
    hxo = nc.declare_dram_parameter("hxo", [n_tiles // ST, P, ST], f32, isOutput=True)

    COLS = F_ATOM * OUT          # 750
    KPW = F_ATOM * kp            # width of positive block
    KNW = F_ATOM * kn

    mul = mybir.AluOpType.mult
    add = mybir.AluOpType.add
    mx = mybir.AluOpType.max

    with tile.TileContext(nc) as tc:
        with (
            tc.tile_pool(name="const", bufs=1) as cp,
            tc.tile_pool(name="ea", bufs=3) as eap,
            tc.tile_pool(name="xed", bufs=3) as xep,
            tc.tile_pool(name="ps", bufs=2, space="PSUM") as psp,
            tc.tile_pool(name="scr", bufs=2) as scrp,
            tc.tile_pool(name="strip", bufs=2) as stp,
            tc.tile_pool(name="acc", bufs=2) as accp,
        ):
            Wt = cp.tile([F_BOND + 1, COLS], f32)
            nc.sync.dma_start(Wt[:], Wf[:])
            rt = cp.tile([P, F_ATOM], f32)
            nc.sync.dma_start(rt[:], rw1[:])

            # ---- node phase: hx[n] = x[n] . rw1 ----
            for g in range(n_tiles // ST):
                hxs = stp.tile([P, ST], f32, tag="hxs")
                xsl = xep.tile([P, ST * F_ATOM], f32, tag="xsl")
                src = xs[g * NODE_GRAN:(g + 1) * NODE_GRAN, :].rearrange(
                    "(t p) f -> p t f", p=P)
                nc.sync.dma_start(
                    xsl[:].rearrange("p (t f) -> p t f", f=F_ATOM), src)
                for c in range(ST):
                    so = scrp.tile([P, F_ATOM], f32, tag="so")
                    nc.vector.tensor_tensor_reduce(
                        out=so[:],
                        in0=xsl[:, c * F_ATOM:(c + 1) * F_ATOM],
                        in1=rt[:],
                        scale=1.0,
                        scalar=0.0,
                        op0=mul,
                        op1=add,
                        accum_out=hxs[:, c:c + 1],
                    )
                nc.sync.dma_start(hxo[g], hxs[:])

            # ---- edge phase ----
            for g in range(e_tiles // ST):
                m1s = stp.tile([P, ST], f32, tag="m1s")
                esl = eap.tile([F_BOND + 1, ST * P], f32, tag="esl")
                nc.sync.dma_start(esl[:], eaT[:, g * EDGE_GRAN:(g + 1) * EDGE_GRAN])
                xesl = xep.tile([P, ST * 2 * F_ATOM], f32, tag="xesl")
                xsrc = xe[g * EDGE_GRAN:(g + 1) * EDGE_GRAN, :].rearrange(
                    "(t p) f -> p t f", p=P)
                nc.sync.dma_start(
                    xesl[:].rearrange("p (t f) -> p t f", f=2 * F_ATOM), xsrc)
                for c in range(ST):
                    ea_t = esl[:, c * P:(c + 1) * P]
                    xe_t = xesl[:, c * 2 * F_ATOM:(c + 1) * 2 * F_ATOM]
                    q = psp.tile([P, 768], f32, tag="q")
                    nc.tensor.matmul(q[:, 0:512], ea_t, Wt[:, 0:512],
                                     start=True, stop=True)
                    nc.tensor.matmul(q[:, 512:COLS], ea_t, Wt[:, 512:COLS],
                                     start=True, stop=True)
                    po = scrp.tile([P, COLS], f32, tag="po")
                    m1a = accp.tile([P, 1], f32, tag="m1a")
                    m1b = accp.tile([P, 1], f32, tag="m1b")
                    if kp > 0:
                        nc.vector.scalar_tensor_tensor(
                            out=po[:, 0:KPW].rearrange("p (i o) -> p i o", o=kp),
                            in0=q[:, 0:KPW].rearrange("p (i o) -> p i o", o=kp),
                            scalar=0.0,
                            in1=xe_t[:, 0:F_ATOM].broadcast_to([P, F_ATOM, kp]),
                            op0=mx,
                            op1=mul,
                            accum_out=m1a[:],
                        )
                    else:
                        nc.vector.memset(m1a[:], 0.0)
                    if kn > 0:
                        nc.vector.scalar_tensor_tensor(
                            out=po[:, KPW:COLS].rearrange("p (i o) -> p i o", o=kn),
                            in0=q[:, KPW:COLS].rearrange("p (i o) -> p i o", o=kn),
                            scalar=0.0,
                            in1=xe_t[:, F_ATOM:2 * F_ATOM]
                                .broadcast_to([P, F_ATOM, kn]),
                            op0=mx,
                            op1=mul,
                            accum_out=m1b[:],
                        )
                    else:
                        nc.vector.memset(m1b[:], 0.0)
                    nc.scalar.add(m1s[:, c:c + 1], m1a[:], add=m1b[:])
                nc.sync.dma_start(m1o[g], m1s[:])
    return nc


def _prep(x, edge_index, edge_attr, batch, lin_w, lin_b, root_w, conv_b,
          out_w, out_b, G):
    """Host-side sharding + weight folding. Returns per-core input maps and
    metadata for the final combine."""
    E = edge_index.shape[1]
    N = x.shape[0]

    src = edge_index[0].astype(np.int64)
    dst = edge_index[1].astype(np.int64)
    ge = batch[dst]                       # graph of each edge's destination
    perm = np.argsort(ge, kind="stable")
    ge_s = ge[perm]
    src_s = src[perm]
    ea_s = edge_attr[perm]

    ecnt = np.bincount(ge_s, minlength=G)
    ecum = np.concatenate([[0], np.cumsum(ecnt)])
    ncnt = np.bincount(batch, minlength=G)
    ncum = np.concatenate([[0], np.cumsum(ncnt)])

    # split graphs into N_CORES contiguous ranges, balanced by edge count
    gb = [0]
    for c in range(1, N_CORES):
        gb.append(int(np.searchsorted(ecum[1:], E * c / N_CORES)))
    gb.append(G)
    gb = np.array(gb)

    e_rngs = [(int(ecum[gb[c]]), int(ecum[gb[c + 1]])) for c in range(N_CORES)]
    n_rngs = [(int(ncum[gb[c]]), int(ncum[gb[c + 1]])) for c in range(N_CORES)]

    max_e = max(e1 - e0 for e0, e1 in e_rngs)
    max_n = max(n1 - n0 for n0, n1 in n_rngs)
    EPC = -(-max_e // EDGE_GRAN) * EDGE_GRAN
    NPC = -(-max_n // NODE_GRAN) * NODE_GRAN

    # weight folding: |ow| into rows, sign via column blocks, i-major o-minor
    ow = out_w.reshape(-1).astype(np.float64)
    o_pos = np.where(ow >= 0)[0]
    o_neg = np.where(ow < 0)[0]
    kp, kn = len(o_pos), len(o_neg)
    o_order = np.concatenate([o_pos, o_neg]).astype(np.int64)
    # column j of block: (i, o) i-major within each sign block
    i_idx = np.repeat(np.arange(F_ATOM), kp)
    o_idx = np.tile(o_pos, F_ATOM)
    rows_p = i_idx * OUT + o_idx
    i_idx = np.repeat(np.arange(F_ATOM), kn)
    o_idx = np.tile(o_neg, F_ATOM)
    rows_n = i_idx * OUT + o_idx
    rows = np.concatenate([rows_p, rows_n])
    absow = np.abs(ow)[np.concatenate([np.tile(o_pos, F_ATOM),
                                       np.tile(o_neg, F_ATOM)])]
    Wcols = lin_w[rows].astype(np.float64) * absow[:, None]          # [750,12]
    bcols = lin_b[rows].astype(np.float64) * absow                   # [750]
    Wf = np.concatenate([Wcols, bcols[:, None]], axis=1).T           # [13,750]
    Wf = np.ascontiguousarray(Wf, dtype=np.float32)

    rw1 = (root_w.astype(np.float64) @ ow).astype(np.float32)        # [75]
    rw1_rep = np.ascontiguousarray(np.broadcast_to(rw1[None, :], (P, F_ATOM)),
                                   dtype=np.float32)

    in_maps = []
    for c in range(N_CORES):
        e0, e1 = e_rngs[c]
        ne = e1 - e0
        eaT = np.zeros((F_BOND + 1, EPC), dtype=np.float32)
        eaT[:F_BOND, :ne] = ea_s[e0:e1].T
        eaT[F_BOND, :ne] = 1.0
        xsrc = x[src_s[e0:e1]].astype(np.float32)
        xef = np.zeros((EPC, 2 * F_ATOM), dtype=np.float32)
        xef[:ne, :F_ATOM] = xsrc
        xef[:ne, F_ATOM:] = -xsrc
        n0, n1 = n_rngs[c]
        nn = n1 - n0
        xsf = np.zeros((NPC, F_ATOM), dtype=np.float32)
        xsf[:nn] = x[n0:n1]
        in_maps.append({
            "eaT": eaT, "xe": xef, "xs": xsf, "Wf": Wf, "rw1": rw1_rep,
        })

    cb1 = float(np.dot(conv_b.astype(np.float64), ow))
    ob = float(np.asarray(out_b).reshape(-1)[0])
    meta = dict(gb=gb, e_rngs=e_rngs, n_rngs=n_rngs, ge_s=ge_s, batch=batch,
                ncnt=ncnt, cb1=cb1, ob=ob, EPC=EPC, NPC=NPC, kp=kp, kn=kn)
    return in_maps, meta


def _combine(results, meta, G):
    gb = meta["gb"]
    out = np.zeros(G, dtype=np.float64)
    for c in range(N_CORES):
        g0, g1 = int(gb[c]), int(gb[c + 1])
        e0, e1 = meta["e_rngs"][c]
        n0, n1 = meta["n_rngs"][c]
        m1 = results[c]["m1o"].transpose(0, 2, 1).reshape(-1)[:e1 - e0]
        hx = results[c]["hxo"].transpose(0, 2, 1).reshape(-1)[:n1 - n0]
        esum = np.bincount(meta["ge_s"][e0:e1] - g0, weights=m1,
                           minlength=g1 - g0)
        nsum = np.bincount(meta["batch"][n0:n1] - g0, weights=hx,
                           minlength=g1 - g0)
        out[g0:g1] = esum + nsum
    out += meta["ncnt"] * meta["cb1"] + meta["ob"]
    return out.astype(np.float32)[:, None]


def kernel(x, edge_index, edge_attr, batch, lin_w, lin_b, root_w, conv_b,
           out_w, out_b, num_graphs, _trace=False):
    from concourse.bass_utils import run_bass_kernel_spmd

    x = np.asarray(x, dtype=np.float32)
    edge_index = np.asarray(edge_index)
    edge_attr = np.asarray(edge_attr, dtype=np.float32)
    batch = np.asarray(batch).astype(np.int64)
    lin_w = np.asarray(lin_w, dtype=np.float32)
    lin_b = np.asarray(lin_b, dtype=np.float32)
    root_w = np.asarray(root_w, dtype=np.float32)
    conv_b = np.asarray(conv_b, dtype=np.float32)
    out_w = np.asarray(out_w, dtype=np.float32)
    out_b = np.asarray(out_b, dtype=np.float32)
    G = int(num_graphs)

    in_maps, meta = _prep(x, edge_index, edge_attr, batch, lin_w, lin_b,
                          root_w, conv_b, out_w, out_b, G)

    key = (meta["EPC"] // P, meta["NPC"] // P, meta["kp"], meta["kn"])
    if key not in _PROG_CACHE:
        _PROG_CACHE[key] = _build_program(*key)
    nc = _PROG_CACHE[key]

    res = run_bass_kernel_spmd(nc, in_maps, list(range(N_CORES)),
                               trace=_trace)
    out = _combine(res.results, meta, G)
    if _trace:
        return out, res
    return out
